# revision 10
# baseline (speedup 1.0000x reference)
"""Trainium2 Bass kernel for causal multi-head attention.

Problem: x[2, 2048, 1024], W_Q/W_K/W_V/W_O [1024, 1024], 16 heads, d_k=64,
causal softmax attention, fp32.

Sharding (8 cores): core c owns batch b=c//4 and head-group g=c%4 (4 heads,
256 cols of W_Q/K/V, 256 rows of W_O). Each core computes a full [S, D]
partial output (its heads' contribution through W_O); host sums the 4
partials per batch.

Device-side per core (all matmuls float32r = fp32 rounded to 11 mantissa
bits, full PE speed at free-dim>=256):
  1. QT/KT/VT = (x @ W)^T via matmuls with W chunks stationary, x^T moving
     (x^T prepared host-side).
  2. V' tiles [128, 65]: V natural layout (PE transpose of VT) + ones column
     (so attn@V also produces softmax denominators for free).
  3. Per (head, q-tile of 512): scores^T[k, q] = K^T-chunk.T @ Q^T (k on
     partitions -> no transpose of probs needed), exp on ScalarE with
     scale=1/8 folded in, causal triangle masked by elementwise multiply,
     attnV: A[65, 512] += V'[kc].T @ E[kc] accumulating over k-chunks.
     Row 64 of A = sum_k exp = softmax denominator.
  4. Normalize: reciprocal_approx_fast on denom row, broadcast via
     ones-matmul, multiply -> NT_h [64, S] normalized out^T per head.
  5. partial^T[e, s] = sum_h W_O[h-rows].T-chunk @ NT_h -> DMA out.
"""

import numpy as np
from contextlib import ExitStack

import concourse.bass as bass
import concourse.tile as tile
from concourse import bacc, mybir
from concourse.bass_utils import run_bass_kernel_spmd

dt = mybir.dt
AF = mybir.ActivationFunctionType

B, S, D, NH, DK = 2, 2048, 1024, 16, 64
NCORES = 8
HPC = 4            # heads per core
CW = HPC * DK      # 256 per-core col width of W_Q/K/V (rows of W_O)
QT_W = 512         # q-tile width
KC_W = 128         # k-chunk width
NQT = S // QT_W    # 4
NKC = S // KC_W    # 16
NDC = D // 128     # 8 contraction chunks for projections
NEC = D // 128     # 8 output-row chunks for W_O stage


def _round_f32r(a: np.ndarray) -> np.ndarray:
    """Round fp32 to f32r (11 mantissa bits, round-half-up) host-side."""
    b = np.ascontiguousarray(a, dtype=np.float32).view(np.uint32)
    b = (b + np.uint32(0x800)) & np.uint32(0xFFFFF000)
    return b.view(np.float32)


def build(debug=False):
    nc = bacc.Bacc("TRN2", target_bir_lowering=False, debug=False,
                   num_devices=NCORES)

    xt_d = nc.dram_tensor("xt", [D, S], dt.float32r, kind="ExternalInput").ap()
    wq_d = nc.dram_tensor("wq", [D, CW], dt.float32r, kind="ExternalInput").ap()
    wk_d = nc.dram_tensor("wk", [D, CW], dt.float32r, kind="ExternalInput").ap()
    wv_d = nc.dram_tensor("wv", [D, CW], dt.float32r, kind="ExternalInput").ap()
    wo_d = nc.dram_tensor("wo", [CW, D], dt.float32r, kind="ExternalInput").ap()
    id_d = nc.dram_tensor("ident", [128, DK], dt.float32r, kind="ExternalInput").ap()
    on_d = nc.dram_tensor("ones", [DK + 1, DK], dt.float32r, kind="ExternalInput").ap()
    tri_d = nc.dram_tensor("tri", [KC_W, KC_W], dt.float32r, kind="ExternalInput").ap()
    vo_d = nc.dram_tensor("vones", [128, NKC * (DK + 1)], dt.float32r,
                          kind="ExternalInput").ap()
    o_d = nc.dram_tensor("o", [D, S], dt.float32, kind="ExternalOutput").ap()
    dbg = {}
    if debug:
        for nm, shp, dty in (("dbg_qt", [128, S], dt.float32r),
                             ("dbg_kt", [128, S], dt.float32r),
                             ("dbg_vt", [128, S], dt.float32r),
                             ("dbg_vp", [128, NKC * (DK + 1)], dt.float32r),
                             ("dbg_e", [128, 4 * QT_W], dt.float32r),
                             ("dbg_os", [DK + 1, S], dt.float32),
                             ("dbg_rh", [DK + 1, S], dt.float32r),
                             ("dbg_nt", [DK, S], dt.float32r)):
            dbg[nm] = nc.dram_tensor(nm, shp, dty, kind="ExternalOutput").ap()

    ts = bass.ts

    with tile.TileContext(nc) as tc, ExitStack() as top:
        # ---- pools that live for the whole kernel ----
        p_const = top.enter_context(tc.tile_pool(name="const", bufs=3))
        p_wo = top.enter_context(tc.tile_pool(name="wo", bufs=HPC))
        p_qt = top.enter_context(tc.tile_pool(name="qt", bufs=2))
        p_kt = top.enter_context(tc.tile_pool(name="kt", bufs=2))
        p_vp = top.enter_context(tc.tile_pool(name="vp", bufs=HPC))

        ident = p_const.tile([128, DK], dt.float32r, name="ident", tag="ident")
        nc.sync.dma_start(out=ident[:], in_=id_d[:])
        ones = p_const.tile([DK + 1, DK], dt.float32r, name="ones", tag="ones")
        nc.sync.dma_start(out=ones[:], in_=on_d[:])
        tri = p_const.tile([KC_W, KC_W], dt.float32r, name="tri", tag="tri")
        nc.sync.dma_start(out=tri[:], in_=tri_d[:])

        wo_sb = []
        for h in range(HPC):
            t = p_wo.tile([DK, D], dt.float32r, name="wo", tag="wo")
            nc.sync.dma_start(out=t[:], in_=wo_d[ts(h, DK), :])
            wo_sb.append(t)

        # QT/KT: [256, S] as 2 partition-group tiles; NT: per-head [64, S]
        qt_sb = [p_qt.tile([128, S], dt.float32r, name="qt", tag="qt") for _ in range(2)]
        kt_sb = [p_kt.tile([128, S], dt.float32r, name="kt", tag="kt") for _ in range(2)]
        # V' per head: [128, 16*65]; block kc at cols [65kc, 65kc+64], ones
        # at col 65kc+64
        vp_sb = [p_vp.tile([128, NKC * (DK + 1)], dt.float32r, name="vp", tag="vp")
                 for _ in range(HPC)]
        # ================= Phase A: projections + V' =================
        with tc.tile_pool(name="xt", bufs=NDC) as p_xt, \
             tc.tile_pool(name="w", bufs=3 * NDC) as p_w, \
             tc.tile_pool(name="vt", bufs=2) as p_vt, \
             tc.tile_pool(name="pp", bufs=6, space="PSUM") as p_pp, \
             tc.tile_pool(name="tp", bufs=2, space="PSUM") as p_tp:

            xt_sb = []
            for dc in range(NDC):
                t = p_xt.tile([128, S], dt.float32r, name="xt", tag="xt")
                nc.sync.dma_start(out=t[:], in_=xt_d[ts(dc, 128), :])
                xt_sb.append(t)

            w_sb = {}
            for mat, wd in (("q", wq_d), ("k", wk_d), ("v", wv_d)):
                w_sb[mat] = []
                for dc in range(NDC):
                    t = p_w.tile([128, CW], dt.float32r, name="w", tag="w")
                    nc.sync.dma_start(out=t[:], in_=wd[ts(dc, 128), :])
                    w_sb[mat].append(t)

            vt_sb = [p_vt.tile([128, S], dt.float32r, name="vt", tag="vt") for _ in range(2)]

            # ones columns of V' (overwritten at cols [65kc, 65kc+64) below)
            for h in range(HPC):
                nc.sync.dma_start(out=vp_sb[h][:], in_=vo_d[:])

            dests = {"q": qt_sb, "k": kt_sb, "v": vt_sb}
            for mat in ("q", "k", "v"):
                for pg in range(2):
                    pp = [p_pp.tile([128, QT_W], dt.float32, name="pp", tag="pp") for _ in range(NQT)]
                    for dc in range(NDC):
                        for st in range(NQT):
                            nc.tensor.matmul(
                                pp[st][:],
                                w_sb[mat][dc][:, ts(pg, 128)],
                                xt_sb[dc][:, ts(st, QT_W)],
                                start=(dc == 0), stop=(dc == NDC - 1),
                            )
                    for st in range(NQT):
                        dst = dests[mat][pg][:, ts(st, QT_W)]
                        if mat == "v":
                            nc.vector.tensor_copy(dst, pp[st][:])
                        else:
                            nc.scalar.copy(dst, pp[st][:])

            if debug:
                nc.sync.dma_start(out=dbg["dbg_qt"][:], in_=qt_sb[0][:])
                nc.sync.dma_start(out=dbg["dbg_kt"][:], in_=kt_sb[0][:])
                nc.sync.dma_start(out=dbg["dbg_vt"][:], in_=vt_sb[0][:])

            # V' build: PE-transpose VT [64, 128] blocks -> [128, 64]
            for h in range(HPC):
                pg, e = h // 2, h % 2
                for kc in range(NKC):
                    tp = p_tp.tile([KC_W, DK], dt.float32r, name="tp", tag="tp")
                    nc.tensor.transpose(
                        tp[:],
                        vt_sb[pg][e * DK:(e + 1) * DK, ts(kc, KC_W)],
                        ident[e * DK:(e + 1) * DK, :],
                    )
                    nc.vector.tensor_copy(
                        vp_sb[h][:, kc * (DK + 1):kc * (DK + 1) + DK], tp[:])

            if debug:
                nc.sync.dma_start(out=dbg["dbg_vp"][:], in_=vp_sb[0][:])

        # ================= Phase B+C scope =================
        p_nt = top.enter_context(tc.tile_pool(name="nt", bufs=HPC))
        nt_sb = [p_nt.tile([DK, S], dt.float32r, name="nt", tag="nt")
                 for _ in range(HPC)]

        # ================= Phase B: attention =================
        with tc.tile_pool(name="e", bufs=4) as p_e, \
             tc.tile_pool(name="os", bufs=2) as p_os, \
             tc.tile_pool(name="rc", bufs=1) as p_rc, \
             tc.tile_pool(name="rh", bufs=1) as p_rh, \
             tc.tile_pool(name="s", bufs=1, space="PSUM") as p_s, \
             tc.tile_pool(name="a", bufs=2, space="PSUM") as p_a, \
             tc.tile_pool(name="bc", bufs=2, space="PSUM") as p_bc:

            for h in range(HPC):
                pg, e = h // 2, h % 2
                prow = slice(e * DK, (e + 1) * DK)
                os_h = p_os.tile([DK + 1, S], dt.float32, name="os", tag="os")
                for qt in range(NQT):
                    a_ps = p_a.tile([DK + 1, QT_W], dt.float32, name="a", tag="a")
                    for g in range(qt + 1):
                        s_ps = p_s.tile([128, 4 * QT_W], dt.float32, name="s", tag="s")
                        e_sb = p_e.tile([128, 4 * QT_W], dt.float32r, name="e", tag="e")
                        diag = (g == qt)
                        for r in range(4):
                            kc = 4 * g + r
                            nc.tensor.matmul(
                                s_ps[:, ts(r, QT_W)],
                                kt_sb[pg][prow, ts(kc, KC_W)],
                                qt_sb[pg][prow, ts(qt, QT_W)],
                                start=True, stop=True,
                            )
                        if not diag:
                            nc.scalar.activation(
                                e_sb[:], s_ps[:], AF.Exp, scale=0.125)
                            if debug and h == 0 and qt == 3 and g == 0:
                                nc.sync.dma_start(out=dbg["dbg_e"][:], in_=e_sb[:])
                        else:
                            for r in range(4):
                                lo = r * QT_W + r * KC_W
                                hi = (r + 1) * QT_W
                                nc.scalar.activation(
                                    e_sb[:, lo:hi], s_ps[:, lo:hi],
                                    AF.Exp, scale=0.125)
                                # causal triangle on first 128 cols of span
                                nc.vector.tensor_mul(
                                    e_sb[:, lo:lo + KC_W],
                                    e_sb[:, lo:lo + KC_W],
                                    tri[:],
                                )
                        for r in range(4):
                            kc = 4 * g + r
                            first = (g == 0 and r == 0)
                            last = (diag and r == 3)
                            if diag:
                                lo_q = r * KC_W
                                nc.tensor.matmul(
                                    a_ps[:, lo_q:QT_W],
                                    vp_sb[h][:, kc * (DK + 1):(kc + 1) * (DK + 1)],
                                    e_sb[:, r * QT_W + lo_q:(r + 1) * QT_W],
                                    start=first, stop=last,
                                )
                            else:
                                nc.tensor.matmul(
                                    a_ps[:],
                                    vp_sb[h][:, kc * (DK + 1):(kc + 1) * (DK + 1)],
                                    e_sb[:, ts(r, QT_W)],
                                    start=first, stop=last,
                                )
                    nc.vector.tensor_copy(os_h[:, ts(qt, QT_W)], a_ps[:])
                    if debug and h == 0 and qt == 3:
                        nc.sync.dma_start(out=dbg["dbg_os"][:], in_=os_h[:])

                # normalization for head h
                # NB: reciprocal_approx_fast silently misbehaves on
                # partition-sliced APs on HW -- run it full-tile.
                rc = p_rc.tile([DK + 1, S], dt.float32, name="rc", tag="rc")
                rh = p_rh.tile([DK + 1, S], dt.float32r, name="rh", tag="rh")
                nc.vector.reciprocal_approx_fast(out=rc[:], in_=os_h[:])
                nc.vector.tensor_copy(rh[DK:DK + 1, :], rc[DK:DK + 1, :])
                if debug and h == 0:
                    nc.sync.dma_start(out=dbg["dbg_rh"][:], in_=rh[:])
                for qt in range(NQT):
                    bc = p_bc.tile([DK, QT_W], dt.float32, name="bc", tag="bc")
                    nc.tensor.matmul(
                        bc[:], ones[DK:DK + 1, :], rh[DK:DK + 1, ts(qt, QT_W)],
                        start=True, stop=True,
                    )
                    nc.vector.tensor_mul(
                        nt_sb[h][:, ts(qt, QT_W)],
                        os_h[0:DK, ts(qt, QT_W)],
                        bc[:],
                    )

        if debug:
            nc.sync.dma_start(out=dbg["dbg_nt"][:], in_=nt_sb[0][:])

        # ================= Phase C: W_O partial =================
        with tc.tile_pool(name="pt", bufs=8, space="PSUM") as p_pt, \
             tc.tile_pool(name="oc", bufs=4) as p_oc:
            for ec in range(NEC):
                pt = [p_pt.tile([128, QT_W], dt.float32, name="pt", tag="pt") for _ in range(NQT)]
                for h in range(HPC):
                    for st in range(NQT):
                        nc.tensor.matmul(
                            pt[st][:],
                            wo_sb[h][:, ts(ec, 128)],
                            nt_sb[h][:, ts(st, QT_W)],
                            start=(h == 0), stop=(h == HPC - 1),
                        )
                for st in range(NQT):
                    oc = p_oc.tile([128, QT_W], dt.float32, name="oc", tag="oc")
                    if st % 2 == 0:
                        nc.vector.tensor_copy(oc[:], pt[st][:])
                    else:
                        nc.scalar.copy(oc[:], pt[st][:])
                    nc.sync.dma_start(
                        out=o_d[ts(ec, 128), ts(st, QT_W)], in_=oc[:])

    nc.compile()
    return nc


_NC = None


def _get_nc():
    global _NC
    if _NC is None:
        _NC = build()
    return _NC


def make_in_maps(x, W_Q, W_K, W_V, W_O):
    x = np.asarray(x, np.float32)
    W_Q, W_K, W_V, W_O = (np.asarray(w, np.float32) for w in (W_Q, W_K, W_V, W_O))
    ident = np.concatenate([np.eye(DK, dtype=np.float32)] * 2, axis=0)
    ones = np.ones((DK + 1, DK), np.float32)
    tri = (np.arange(KC_W)[:, None] <= np.arange(KC_W)[None, :]).astype(np.float32)
    vones = np.ones((128, NKC * (DK + 1)), np.float32)
    in_maps = []
    for c in range(NCORES):
        b, g = c // HPC, c % HPC
        cols = slice(g * CW, (g + 1) * CW)
        in_maps.append({
            "xt": _round_f32r(x[b].T),
            "wq": _round_f32r(W_Q[:, cols]),
            "wk": _round_f32r(W_K[:, cols]),
            "wv": _round_f32r(W_V[:, cols]),
            "wo": _round_f32r(W_O[cols, :]),
            "ident": ident,
            "ones": ones,
            "tri": tri,
            "vones": vones,
        })
    return in_maps


def gather_output(results):
    out = np.zeros((B, S, D), np.float32)
    for c in range(NCORES):
        out[c // HPC] += results[c]["o"].T
    return out


def kernel(x, W_Q, W_K, W_V, W_O):
    nc = _get_nc()
    res = run_bass_kernel_spmd(
        nc, make_in_maps(x, W_Q, W_K, W_V, W_O), list(range(NCORES))).results
    return gather_output(res)


# revision 13
# speedup vs baseline: 1.0853x; 1.0853x over previous
"""Trainium2 Bass kernel for causal multi-head attention.

Problem: x[2, 2048, 1024], W_Q/W_K/W_V/W_O [1024, 1024], 16 heads, d_k=64,
causal softmax attention, fp32.

Sharding (8 cores): core c owns batch b=c//4 and head-group g=c%4 (4 heads,
256 cols of W_Q/K/V, 256 rows of W_O). Each core computes a full [S, D]
partial output (its heads' contribution through W_O); host sums the 4
partials per batch.

Device-side per core (all matmuls float32r = fp32 rounded to 11 mantissa
bits, full PE speed at free-dim>=256):
  1. QT/KT/VT = (x @ W)^T via matmuls with W chunks stationary, x^T moving
     (x^T prepared host-side).
  2. V' tiles [128, 65]: V natural layout (PE transpose of VT) + ones column
     (so attn@V also produces softmax denominators for free).
  3. Per (head, q-tile of 512): scores^T[k, q] = K^T-chunk.T @ Q^T (k on
     partitions -> no transpose of probs needed), exp on ScalarE with
     scale=1/8 folded in, causal triangle masked by elementwise multiply,
     attnV: A[65, 512] += V'[kc].T @ E[kc] accumulating over k-chunks.
     Row 64 of A = sum_k exp = softmax denominator.
  4. Normalize: reciprocal_approx_fast on denom row, broadcast via
     ones-matmul, multiply -> NT_h [64, S] normalized out^T per head.
  5. partial^T[e, s] = sum_h W_O[h-rows].T-chunk @ NT_h -> DMA out.
"""

import numpy as np
from contextlib import ExitStack

import concourse.bass as bass
import concourse.tile as tile
from concourse import bacc, mybir
from concourse.bass_utils import run_bass_kernel_spmd

dt = mybir.dt
AF = mybir.ActivationFunctionType

B, S, D, NH, DK = 2, 2048, 1024, 16, 64
NCORES = 8
HPC = 4            # heads per core
CW = HPC * DK      # 256 per-core col width of W_Q/K/V (rows of W_O)
QT_W = 512         # q-tile width
KC_W = 128         # k-chunk width
NQT = S // QT_W    # 4
NKC = S // KC_W    # 16
NDC = D // 128     # 8 contraction chunks for projections
NEC = D // 128     # 8 output-row chunks for W_O stage


def _round_f32r(a: np.ndarray) -> np.ndarray:
    """Round fp32 to f32r (11 mantissa bits, round-half-up) host-side."""
    b = np.ascontiguousarray(a, dtype=np.float32).view(np.uint32)
    b = (b + np.uint32(0x800)) & np.uint32(0xFFFFF000)
    return b.view(np.float32)


def build(debug=False):
    nc = bacc.Bacc("TRN2", target_bir_lowering=False, debug=False,
                   num_devices=NCORES)

    xt_d = nc.dram_tensor("xt", [D, S], dt.float32r, kind="ExternalInput").ap()
    wq_d = nc.dram_tensor("wq", [D, CW], dt.float32r, kind="ExternalInput").ap()
    wk_d = nc.dram_tensor("wk", [D, CW], dt.float32r, kind="ExternalInput").ap()
    wv_d = nc.dram_tensor("wv", [D, CW], dt.float32r, kind="ExternalInput").ap()
    wo_d = nc.dram_tensor("wo", [CW, D], dt.float32r, kind="ExternalInput").ap()
    id_d = nc.dram_tensor("ident", [128, DK], dt.float32r, kind="ExternalInput").ap()
    on_d = nc.dram_tensor("ones", [DK + 1, DK], dt.float32r, kind="ExternalInput").ap()
    tri_d = nc.dram_tensor("tri", [KC_W, KC_W], dt.float32r, kind="ExternalInput").ap()
    vo_d = nc.dram_tensor("vones", [128, NKC * (DK + 1)], dt.float32r,
                          kind="ExternalInput").ap()
    o_d = nc.dram_tensor("o", [D, S], dt.float32, kind="ExternalOutput").ap()
    dbg = {}
    if debug:
        for nm, shp, dty in (("dbg_qt", [128, S], dt.float32r),
                             ("dbg_kt", [128, S], dt.float32r),
                             ("dbg_vt", [128, S], dt.float32r),
                             ("dbg_vp", [128, NKC * (DK + 1)], dt.float32r),
                             ("dbg_e", [128, 4 * QT_W], dt.float32r),
                             ("dbg_os", [DK + 1, S], dt.float32),
                             ("dbg_rh", [DK + 1, S], dt.float32r),
                             ("dbg_nt", [DK, S], dt.float32r)):
            dbg[nm] = nc.dram_tensor(nm, shp, dty, kind="ExternalOutput").ap()

    ts = bass.ts

    with tile.TileContext(nc) as tc, ExitStack() as top:
        # ---- pools that live for the whole kernel ----
        p_const = top.enter_context(tc.tile_pool(name="const", bufs=3))
        p_wo = top.enter_context(tc.tile_pool(name="wo", bufs=HPC))
        p_qt = top.enter_context(tc.tile_pool(name="qt", bufs=2))
        p_kt = top.enter_context(tc.tile_pool(name="kt", bufs=2))
        p_vp = top.enter_context(tc.tile_pool(name="vp", bufs=HPC))

        ident = p_const.tile([128, DK], dt.float32r, name="ident", tag="ident")
        nc.sync.dma_start(out=ident[:], in_=id_d[:])
        ones = p_const.tile([DK + 1, DK], dt.float32r, name="ones", tag="ones")
        nc.sync.dma_start(out=ones[:], in_=on_d[:])
        tri = p_const.tile([KC_W, KC_W], dt.float32r, name="tri", tag="tri")
        nc.sync.dma_start(out=tri[:], in_=tri_d[:])

        wo_sb = []
        for h in range(HPC):
            t = p_wo.tile([DK, D], dt.float32r, name="wo", tag="wo")
            nc.sync.dma_start(out=t[:], in_=wo_d[ts(h, DK), :])
            wo_sb.append(t)

        # QT/KT: [256, S] as 2 partition-group tiles; NT: per-head [64, S]
        qt_sb = [p_qt.tile([128, S], dt.float32r, name="qt", tag="qt") for _ in range(2)]
        kt_sb = [p_kt.tile([128, S], dt.float32r, name="kt", tag="kt") for _ in range(2)]
        # V' per head: [128, 16*65]; block kc at cols [65kc, 65kc+64], ones
        # at col 65kc+64
        vp_sb = [p_vp.tile([128, NKC * (DK + 1)], dt.float32r, name="vp", tag="vp")
                 for _ in range(HPC)]
        # ================= Phase A: projections + V' =================
        with tc.tile_pool(name="xt", bufs=NDC) as p_xt, \
             tc.tile_pool(name="w", bufs=3 * NDC) as p_w, \
             tc.tile_pool(name="vt", bufs=2) as p_vt, \
             tc.tile_pool(name="pp", bufs=6, space="PSUM") as p_pp, \
             tc.tile_pool(name="tp", bufs=2, space="PSUM") as p_tp:

            w_sb = {}
            for mat, wd in (("q", wq_d), ("k", wk_d), ("v", wv_d)):
                w_sb[mat] = []
                for dc in range(NDC):
                    t = p_w.tile([128, CW], dt.float32r, name="w", tag="w")
                    nc.gpsimd.dma_start(out=t[:], in_=wd[ts(dc, 128), :])
                    w_sb[mat].append(t)

            # xt loaded st-major so the first psum group's inputs land first
            xt_sb = [p_xt.tile([128, S], dt.float32r, name="xt", tag="xt")
                     for _ in range(NDC)]
            for st in range(NQT):
                for dc in range(NDC):
                    nc.sync.dma_start(out=xt_sb[dc][:, ts(st, QT_W)],
                                      in_=xt_d[ts(dc, 128), ts(st, QT_W)])

            vt_sb = [p_vt.tile([128, S], dt.float32r, name="vt", tag="vt") for _ in range(2)]

            # ones columns of V' (overwritten at cols [65kc, 65kc+64) below)
            for h in range(HPC):
                nc.scalar.dma_start(out=vp_sb[h][:], in_=vo_d[:])

            dests = {"q": qt_sb, "k": kt_sb, "v": vt_sb}
            for mat in ("q", "k", "v"):
                for pg in range(2):
                    pp = [p_pp.tile([128, QT_W], dt.float32, name="pp", tag="pp") for _ in range(NQT)]
                    for dc in range(NDC):
                        for st in range(NQT):
                            nc.tensor.matmul(
                                pp[st][:],
                                w_sb[mat][dc][:, ts(pg, 128)],
                                xt_sb[dc][:, ts(st, QT_W)],
                                start=(dc == 0), stop=(dc == NDC - 1),
                            )
                    for st in range(NQT):
                        dst = dests[mat][pg][:, ts(st, QT_W)]
                        if mat == "v":
                            nc.vector.tensor_copy(dst, pp[st][:])
                        else:
                            nc.scalar.copy(dst, pp[st][:])

            if debug:
                nc.sync.dma_start(out=dbg["dbg_qt"][:], in_=qt_sb[0][:])
                nc.sync.dma_start(out=dbg["dbg_kt"][:], in_=kt_sb[0][:])
                nc.sync.dma_start(out=dbg["dbg_vt"][:], in_=vt_sb[0][:])

            # V' build: PE-transpose VT [64, 128] blocks -> [128, 64]
            for h in range(HPC):
                pg, e = h // 2, h % 2
                for kc in range(NKC):
                    tp = p_tp.tile([KC_W, DK], dt.float32r, name="tp", tag="tp")
                    nc.tensor.transpose(
                        tp[:],
                        vt_sb[pg][e * DK:(e + 1) * DK, ts(kc, KC_W)],
                        ident[e * DK:(e + 1) * DK, :],
                    )
                    nc.vector.tensor_copy(
                        vp_sb[h][:, kc * (DK + 1):kc * (DK + 1) + DK], tp[:])

            if debug:
                nc.sync.dma_start(out=dbg["dbg_vp"][:], in_=vp_sb[0][:])

        # ================= Phase B+C scope =================
        p_nt = top.enter_context(tc.tile_pool(name="nt", bufs=HPC))
        nt_sb = [p_nt.tile([DK, S], dt.float32r, name="nt", tag="nt")
                 for _ in range(HPC)]

        # ================= Phase B: attention =================
        with tc.tile_pool(name="e", bufs=6) as p_e, \
             tc.tile_pool(name="os", bufs=2) as p_os, \
             tc.tile_pool(name="rc", bufs=1) as p_rc, \
             tc.tile_pool(name="rh", bufs=1) as p_rh, \
             tc.tile_pool(name="s", bufs=2, space="PSUM") as p_s, \
             tc.tile_pool(name="a", bufs=2, space="PSUM") as p_a, \
             tc.tile_pool(name="bc", bufs=2, space="PSUM") as p_bc:

            def attn_unit(h, qt, os_h):
                """scores+exp+attnV for one (head, q-tile): A -> os_h slice."""
                pg, e = h // 2, h % 2
                prow = slice(e * DK, (e + 1) * DK)
                a_ps = p_a.tile([DK + 1, QT_W], dt.float32, name="a", tag="a")
                nkc = 4 * (qt + 1)
                first = True
                for g2 in range(nkc // 2):
                    kcs = [2 * g2, 2 * g2 + 1]
                    s_ps = p_s.tile([128, 2 * QT_W], dt.float32, name="s", tag="s")
                    e_sb = p_e.tile([128, 2 * QT_W], dt.float32r, name="e", tag="e")
                    for j, kc in enumerate(kcs):
                        nc.tensor.matmul(
                            s_ps[:, ts(j, QT_W)],
                            kt_sb[pg][prow, ts(kc, KC_W)],
                            qt_sb[pg][prow, ts(qt, QT_W)],
                            start=True, stop=True,
                        )
                    if kcs[1] < 4 * qt:  # fully sub-diagonal group
                        nc.scalar.activation(
                            e_sb[:], s_ps[:], AF.Exp, scale=0.125)
                    else:
                        for j, kc in enumerate(kcs):
                            r = kc - 4 * qt
                            if r < 0:  # sub-diagonal kc within mixed group
                                nc.scalar.activation(
                                    e_sb[:, ts(j, QT_W)], s_ps[:, ts(j, QT_W)],
                                    AF.Exp, scale=0.125)
                                continue
                            lo = j * QT_W + r * KC_W
                            hi = (j + 1) * QT_W
                            nc.scalar.activation(
                                e_sb[:, lo:hi], s_ps[:, lo:hi],
                                AF.Exp, scale=0.125)
                            nc.vector.tensor_mul(
                                e_sb[:, lo:lo + KC_W],
                                e_sb[:, lo:lo + KC_W],
                                tri[:],
                            )
                    for j, kc in enumerate(kcs):
                        r = kc - 4 * qt
                        last = (kc == nkc - 1)
                        if r > 0:
                            lo_q = r * KC_W
                            nc.tensor.matmul(
                                a_ps[:, lo_q:QT_W],
                                vp_sb[h][:, kc * (DK + 1):(kc + 1) * (DK + 1)],
                                e_sb[:, j * QT_W + lo_q:(j + 1) * QT_W],
                                start=False, stop=last,
                            )
                        else:
                            nc.tensor.matmul(
                                a_ps[:],
                                vp_sb[h][:, kc * (DK + 1):(kc + 1) * (DK + 1)],
                                e_sb[:, ts(j, QT_W)],
                                start=first, stop=last,
                            )
                            first = False
                nc.vector.tensor_copy(os_h[:, ts(qt, QT_W)], a_ps[:])
                if debug and h == 0 and qt == 3:
                    nc.sync.dma_start(out=dbg["dbg_os"][:], in_=os_h[:])

            def normalize(h, os_h):
                # NB: reciprocal_approx_fast silently misbehaves on
                # partition-sliced APs on HW -- run it full-tile.
                rc = p_rc.tile([DK + 1, S], dt.float32, name="rc", tag="rc")
                rh = p_rh.tile([DK + 1, S], dt.float32r, name="rh", tag="rh")
                nc.vector.reciprocal_approx_fast(out=rc[:], in_=os_h[:])
                nc.vector.tensor_copy(rh[DK:DK + 1, :], rc[DK:DK + 1, :])
                if debug and h == 0:
                    nc.sync.dma_start(out=dbg["dbg_rh"][:], in_=rh[:])
                for qt in range(NQT):
                    bc = p_bc.tile([DK, QT_W], dt.float32, name="bc", tag="bc")
                    nc.tensor.matmul(
                        bc[:], ones[DK:DK + 1, :], rh[DK:DK + 1, ts(qt, QT_W)],
                        start=True, stop=True,
                    )
                    nc.vector.tensor_mul(
                        nt_sb[h][:, ts(qt, QT_W)],
                        os_h[0:DK, ts(qt, QT_W)],
                        bc[:],
                    )

            for hp in range(HPC // 2):
                h0, h1 = 2 * hp, 2 * hp + 1
                os0 = p_os.tile([DK + 1, S], dt.float32, name="os", tag="os")
                os1 = p_os.tile([DK + 1, S], dt.float32, name="os", tag="os")
                for qt in range(NQT):
                    attn_unit(h0, qt, os0)
                    attn_unit(h1, qt, os1)
                normalize(h0, os0)
                normalize(h1, os1)

        if debug:
            nc.sync.dma_start(out=dbg["dbg_nt"][:], in_=nt_sb[0][:])

        # ================= Phase C: W_O partial =================
        with tc.tile_pool(name="pt", bufs=8, space="PSUM") as p_pt, \
             tc.tile_pool(name="oc", bufs=4) as p_oc:
            for ec in range(NEC):
                pt = [p_pt.tile([128, QT_W], dt.float32, name="pt", tag="pt") for _ in range(NQT)]
                for h in range(HPC):
                    for st in range(NQT):
                        nc.tensor.matmul(
                            pt[st][:],
                            wo_sb[h][:, ts(ec, 128)],
                            nt_sb[h][:, ts(st, QT_W)],
                            start=(h == 0), stop=(h == HPC - 1),
                        )
                for st in range(NQT):
                    oc = p_oc.tile([128, QT_W], dt.float32, name="oc", tag="oc")
                    if st % 2 == 0:
                        nc.vector.tensor_copy(oc[:], pt[st][:])
                    else:
                        nc.scalar.copy(oc[:], pt[st][:])
                    nc.sync.dma_start(
                        out=o_d[ts(ec, 128), ts(st, QT_W)], in_=oc[:])

    nc.compile()
    return nc


_NC = None


def _get_nc():
    global _NC
    if _NC is None:
        _NC = build()
    return _NC


def make_in_maps(x, W_Q, W_K, W_V, W_O):
    x = np.asarray(x, np.float32)
    W_Q, W_K, W_V, W_O = (np.asarray(w, np.float32) for w in (W_Q, W_K, W_V, W_O))
    ident = np.concatenate([np.eye(DK, dtype=np.float32)] * 2, axis=0)
    ones = np.ones((DK + 1, DK), np.float32)
    tri = (np.arange(KC_W)[:, None] <= np.arange(KC_W)[None, :]).astype(np.float32)
    vones = np.ones((128, NKC * (DK + 1)), np.float32)
    in_maps = []
    for c in range(NCORES):
        b, g = c // HPC, c % HPC
        cols = slice(g * CW, (g + 1) * CW)
        in_maps.append({
            "xt": _round_f32r(x[b].T),
            "wq": _round_f32r(W_Q[:, cols]),
            "wk": _round_f32r(W_K[:, cols]),
            "wv": _round_f32r(W_V[:, cols]),
            "wo": _round_f32r(W_O[cols, :]),
            "ident": ident,
            "ones": ones,
            "tri": tri,
            "vones": vones,
        })
    return in_maps


def gather_output(results):
    out = np.zeros((B, S, D), np.float32)
    for c in range(NCORES):
        out[c // HPC] += results[c]["o"].T
    return out


def kernel(x, W_Q, W_K, W_V, W_O):
    nc = _get_nc()
    res = run_bass_kernel_spmd(
        nc, make_in_maps(x, W_Q, W_K, W_V, W_O), list(range(NCORES))).results
    return gather_output(res)
